# revision 20
# baseline (speedup 1.0000x reference)
"""AttentionBlock (GroupNorm + single-head self-attention + residual) on 8 TRN2 cores.

Sharding: 8 cores = 4 batch samples x 2 query-halves. Each core receives the
full 4096-token sample with its own half's tokens permuted to the front
(GroupNorm stats, K/V and softmax sums are token-permutation invariant),
computes the block for its 2048 query rows, and writes [2048, 256].

fp8 pipeline (fp8e4 DoubleRow matmuls = 4x the f32r rate on the PE):
  B) load x token-major; GroupNorm sums as in the f32r kernel (DVE partials,
     ACT squares + PE ones-matmul chain)
  C) PE-transpose x to channel-major, drain as fp8 x8 [chan_p, 2, tok];
     GroupNorm affine folded into fp8 QKV weights (w8 = fp8(16*s*W), biases
     b8 = 16*(b + t@W) as per-partition columns)
  D) QKV projections as fp8 DoubleRow matmuls (contraction 256 in one
     instruction); drains write fp8 q8/k8 (channel-major) and v8 (key-pair
     tiles [key_p, 2, chan])
  E) attention per 512-query block: S^T = K'^T Q' per key pair in one
     DoubleRow matmul per 128-key tile; exp on ACT (scale/256, bias -3,
     output fp8); E V and the softmax denominator accumulate over the 16
     pairs in PSUM via DoubleRow matmuls (ones-column for the denominator)
  F) block-end dchain (denominator -> per-token columns -> reciprocal);
     delayed projection epilogue in f32r, divide on DVE, residual on Pool

ACT (exp: 64 tiles of [128,1024]) is the critical engine; PE ~50us.
"""

import numpy as np
from contextlib import ExitStack

import concourse.bass as bass
import concourse.bacc as bacc
import concourse.tile as tile
from concourse import mybir
from concourse.bass_utils import run_bass_kernel_spmd
from concourse.masks import make_identity

F32 = mybir.dt.float32
F32R = mybir.dt.float32r
F8 = mybir.dt.float8e4
AX = mybir.AxisListType.X
AF = mybir.ActivationFunctionType
DR = mybir.MatmulPerfMode.DoubleRow

B, H, W, C = 4, 64, 64, 256
TOK = H * W          # 4096 tokens per sample
NQ = TOK // 2        # 2048 query rows per core
G, GS = 8, C // 8    # groups, group size
EPS = 1e-3
SCALE = float(C) ** -0.5
QS = 16.0            # fp8 q/k/v pre-scale
C_EXP = 3.0          # exp offset: weights scaled e^-3 to fit fp8e4 (max 240)
SCALE8 = SCALE / (QS * QS)
N_CORES = 8
NT = TOK // 128      # 32 token tiles
NQT = NQ // 128      # 16 query token tiles
NB = NQ // 512       # 4 query blocks
CT = C // 128        # 2 channel tiles
NPAIR = NT // 2      # 16 pairs of key tiles per query block


def build_nc(use_f32r=True, reps=1, trace_sim=False):
    mmdt = F32R if use_f32r else F32
    nc = bacc.Bacc(trn_type="TRN2")

    xs_d = nc.declare_dram_parameter("xs", [TOK, C], F32R, isOutput=False)
    wq_d = nc.declare_dram_parameter("Wq", [C, C], mmdt, isOutput=False)
    wk_d = nc.declare_dram_parameter("Wk", [C, C], mmdt, isOutput=False)
    wv_d = nc.declare_dram_parameter("Wv", [C, C], mmdt, isOutput=False)
    wp_d = nc.declare_dram_parameter("Wp", [C, C], mmdt, isOutput=False)
    bq_d = nc.declare_dram_parameter("bq", [C], F32, isOutput=False)
    bk_d = nc.declare_dram_parameter("bk", [C], F32, isOutput=False)
    bv_d = nc.declare_dram_parameter("bv", [C], F32, isOutput=False)
    bp_d = nc.declare_dram_parameter("bp", [C], F32, isOutput=False)
    gam_d = nc.declare_dram_parameter("gn_gamma", [C], F32, isOutput=False)
    bet_d = nc.declare_dram_parameter("gn_beta", [C], F32, isOutput=False)
    out_d = nc.declare_dram_parameter("out", [NQ, C], F32, isOutput=True)

    with tile.TileContext(nc, trace_sim=trace_sim) as tc:
      for _rep in range(reps):
       with ExitStack() as stack:
        consts = stack.enter_context(tc.tile_pool(name="consts", bufs=1))
        persist = stack.enter_context(tc.tile_pool(name="persist", bufs=1))
        dram = stack.enter_context(tc.tile_pool(name="dram", bufs=1, space="DRAM"))

        # ---- Phase A: constants ----
        ident = consts.tile([128, 128], F32)
        make_identity(nc, ident)
        ident_r = consts.tile([128, 128], F32R)
        nc.vector.tensor_copy(ident_r, ident)
        ones = consts.tile([128, 1], F32)
        nc.vector.memset(ones, 1.0)
        ones_r = consts.tile([128, 1], F32R)
        nc.vector.tensor_copy(ones_r, ones)
        ones8 = consts.tile([128, 2, 32], F8)
        nc.vector.memset(ones8, 1.0)
        negc = consts.tile([128, 1], F32)
        nc.vector.memset(negc, -C_EXP)
        expwarm = consts.tile([1, 1], F32)
        nc.scalar.activation(expwarm, negc[0:1, :], AF.Exp, scale=1.0)
        epsc = consts.tile([1, 1], F32)
        nc.vector.memset(epsc, EPS)

        xk = [persist.tile([128, C], F32R, name=f"xk{i}") for i in range(NT)]
        xkf = xk
        for i in range(NT):
            eng = (nc.sync, nc.gpsimd, nc.gpsimd)[i % 3]
            eng.dma_start(out=xk[i], in_=xs_d[i * 128:(i + 1) * 128, :])

        grow = consts.tile([1, C], F32)
        nc.sync.dma_start(out=grow, in_=gam_d[:].rearrange("(a c) -> a c", a=1))
        brow = consts.tile([1, C], F32)
        nc.sync.dma_start(out=brow, in_=bet_d[:].rearrange("(a c) -> a c", a=1))
        bprow = consts.tile([1, C], F32)
        nc.sync.dma_start(out=bprow, in_=bp_d[:].rearrange("(a c) -> a c", a=1))

        wq_t, wk_t, wv_t, wp_t = [], [], [], []
        for kk in range(CT):
            for lst, src, nm in (
                (wq_t, wq_d, "wq"), (wk_t, wk_d, "wk"),
                (wv_t, wv_d, "wv"), (wp_t, wp_d, "wp"),
            ):
                t = consts.tile([128, C], mmdt, name=f"{nm}{kk}")
                nc.sync.dma_start(out=t, in_=src[kk * 128:(kk + 1) * 128, :])
                lst.append(t)
        bvrow = consts.tile([1, C], F32)
        nc.sync.dma_start(out=bvrow, in_=bv_d[:].rearrange("(a c) -> a c", a=1))
        bqc = []
        for m in range(CT):
            tq = consts.tile([128, 1], F32, name=f"bqc{m}")
            nc.sync.dma_start(
                out=tq, in_=bq_d[m * 128:(m + 1) * 128].rearrange("(p a) -> p a", a=1))
            bqc.append(tq)

        # fp8 operands (x8/k8/q8 channel-major [chan_p, chan_tile, tok])
        x8 = persist.tile([128, CT, TOK], F8, name="x8")
        k8 = persist.tile([128, CT, TOK], F8, name="k8")
        q8 = persist.tile([128, CT, NQ], F8, name="q8")
        v8all = persist.tile([128, 2, NPAIR * C], F8, name="v8all")
        w8q = persist.tile([128, CT, C], F8, name="w8q")
        w8k = persist.tile([128, CT, C], F8, name="w8k")
        w8v = persist.tile([128, CT, C], F8, name="w8v")

        # ---- Phase B: GroupNorm statistics ----
        xt = xk
        with (
            tc.tile_pool(name="statp", bufs=1) as statp,
            tc.tile_pool(name="sqp", bufs=3) as sqp,
            tc.tile_pool(name="statps", bufs=1, space="PSUM") as statps,
            tc.tile_pool(name="tps", bufs=3, space="PSUM") as tps,
        ):
            sq_ps = statps.tile([1, C], F32, tag="sqps")
            sum_ps = statps.tile([1, C], F32, tag="sumps")

            def emit_transposes(i0):
                for cc in range(CT):
                    tp = tps.tile([128, 512], F32R, tag="tp")
                    for j in range(4):
                        nc.tensor.transpose(
                            tp[:, j * 128:(j + 1) * 128],
                            xt[i0 + j][:, cc * 128:(cc + 1) * 128], ident_r)
                    nc.vector.tensor_copy(
                        x8[:, cc, i0 * 128:(i0 + 4) * 128], tp)
            for i in range(NT):
                if i % 4 == 0:
                    emit_transposes(i)
                t = xk[i]
                tf = xkf[i]
                sqt = sqp.tile([128, C], mmdt, tag="sq")
                nc.gpsimd.tensor_mul(sqt, tf, tf)
                nc.tensor.matmul(sum_ps, ones_r if use_f32r else ones, tf,
                                 start=(i == 0), stop=(i == NT - 1))
                nc.tensor.matmul(sq_ps, ones_r if use_f32r else ones, sqt,
                                 start=(i == 0), stop=(i == NT - 1))

            # group math: g16 = [rstd_g | mean_g]
            g16 = statp.tile([1, 2 * G], F32)
            meang = g16[:, G:2 * G]
            nc.vector.reduce_sum(
                out=meang, in_=sum_ps.rearrange("a (g d) -> a g d", g=G), axis=AX)
            nc.vector.tensor_scalar_mul(meang, meang, 1.0 / (TOK * GS))
            msqg = statp.tile([1, G], F32)
            nc.vector.reduce_sum(
                out=msqg, in_=sq_ps.rearrange("a (g d) -> a g d", g=G), axis=AX)
            nc.vector.tensor_scalar_mul(msqg, msqg, 1.0 / (TOK * GS))
            m2 = statp.tile([1, G], F32)
            nc.vector.tensor_mul(m2, meang, meang)
            varg = statp.tile([1, G], F32)
            nc.vector.tensor_sub(varg, msqg, m2)
            nc.vector.tensor_scalar_add(varg, varg, EPS)
            # rstd = 1/sqrt(varg): Quake seed + 2 Newton iterations (DVE
            # only, avoids the ACT Sqrt table load)
            I32 = mybir.dt.int32
            yseed = statp.tile([1, G], F32)
            y_i = yseed.bitcast(I32)
            nc.vector.tensor_scalar(
                out=y_i, in0=varg.bitcast(I32), scalar1=1, scalar2=None,
                op0=mybir.AluOpType.logical_shift_right)
            nc.vector.tensor_scalar(
                out=y_i, in0=y_i, scalar1=-1, scalar2=0x5f3759df,
                op0=mybir.AluOpType.mult, op1=mybir.AluOpType.add)
            half_v = statp.tile([1, G], F32)
            nc.vector.tensor_scalar_mul(half_v, varg, 0.5)
            yy = statp.tile([1, G], F32)
            tt = statp.tile([1, G], F32)
            for _ in range(2):
                nc.vector.tensor_mul(yy, yseed, yseed)
                nc.vector.tensor_mul(tt, yy, half_v)
                nc.vector.tensor_scalar(
                    out=tt, in0=tt, scalar1=-1.0, scalar2=1.5,
                    op0=mybir.AluOpType.mult, op1=mybir.AluOpType.add)
                nc.vector.tensor_mul(yseed, yseed, tt)
            nc.vector.tensor_copy(g16[:, 0:G], yseed)

            # expand groups -> channels: step-0 broadcast reads on DVE
            rstd_b = statp.tile([1, C], F32)
            nc.vector.tensor_copy(
                rstd_b.rearrange("a (g d) -> a g d", g=G),
                g16[:, 0:G].rearrange("a (g d) -> a g d", g=G).to_broadcast((1, G, GS)))
            mean_b = statp.tile([1, C], F32)
            nc.vector.tensor_copy(
                mean_b.rearrange("a (g d) -> a g d", g=G),
                g16[:, G:2 * G].rearrange("a (g d) -> a g d", g=G).to_broadcast((1, G, GS)))

            # per-channel scale s and shift t rows
            srow = statp.tile([1, C], F32)
            nc.vector.tensor_mul(srow, rstd_b, grow)
            tmpr = statp.tile([1, C], F32)
            nc.vector.tensor_mul(tmpr, mean_b, srow)
            trow = statp.tile([1, C], F32)
            nc.vector.tensor_sub(trow, brow, tmpr)
            srow16 = statp.tile([1, C], F32)
            nc.vector.tensor_scalar_mul(srow16, srow, QS)

            # scatter s/t rows to DRAM; reload as columns / broadcasts
            sscr = dram.tile([C], F32)
            nc.sync.dma_start(out=sscr, in_=srow)
            tscr = dram.tile([C], F32)
            nc.sync.dma_start(out=tscr, in_=trow)

            def row_to_cols(row, dtype, nm):
                cols = []
                for cc in range(CT):
                    cp = statps.tile([128, 1], F32, tag="colp", bufs=1,
                                     name=f"{nm}p{cc}")
                    nc.tensor.transpose(
                        cp, row[:, cc * 128:(cc + 1) * 128], ident[0:1, 0:1])
                    col = persist.tile([128, 1], dtype, name=f"{nm}{cc}")
                    nc.vector.tensor_copy(col, cp)
                    cols.append(col)
                return cols
            scol16 = row_to_cols(srow16, F32, "scol16")
            tcol = row_to_cols(trow, mmdt, "tcol")
            # fold the normalize into the QKV weights: w8 = fp8(QS*diag(s)W)
            # and b8 = QS*(b + t @ W) via tiny PE matmuls

            # fp8 QKV weights first: they gate the Q/K/V matmuls
            for wt, w8 in ((wq_t, w8q), (wk_t, w8k), (wv_t, w8v)):
                for kk in range(CT):
                    nc.vector.tensor_scalar_mul(w8[:, kk, :], wt[kk], scol16[kk])
            tw_rows = {}
            for nm, wt in (("q", wq_t), ("v", wv_t)):
                twp = statps.tile([1, C], F32, tag="srow", bufs=2, name=f"twp{nm}")
                for kk in range(CT):
                    nc.tensor.matmul(twp, tcol[kk], wt[kk],
                                     start=(kk == 0), stop=(kk == CT - 1))
                twr = statp.tile([1, C], F32, name=f"twr{nm}")
                nc.vector.tensor_copy(twr, twp)
                tw_rows[nm] = twr
            # bvv = bv + t @ Wv, then bvwp = bvv @ Wp for the final bias
            bvv = statp.tile([1, C], F32)
            nc.vector.tensor_add(bvv, tw_rows["v"], bvrow)
            # rows -> DRAM so they can be reloaded as per-partition columns
            twqc = row_to_cols(tw_rows["q"], F32, "twqc")
            bvvc = row_to_cols(bvv, mmdt, "bvvc")
            for cc in range(CT):
                nc.vector.tensor_add(bqc[cc], bqc[cc], twqc[cc])
                nc.vector.tensor_scalar_mul(bqc[cc], bqc[cc], QS)
            bvwp_ps = statps.tile([1, C], F32, tag="srow", bufs=2)
            for kk in range(CT):
                nc.tensor.matmul(bvwp_ps, bvvc[kk], wp_t[kk],
                                 start=(kk == 0), stop=(kk == CT - 1))
            tfin = statp.tile([1, C], F32)
            nc.vector.tensor_copy(tfin, bvwp_ps)
            nc.vector.tensor_add(tfin, tfin, trow)
            nc.vector.tensor_add(tfin, tfin, bprow)
            tfscr = dram.tile([C], F32)
            nc.sync.dma_start(out=tfscr, in_=tfin)

            s_bcast = persist.tile([128, C], F32)
            nc.gpsimd.dma_start(
                out=s_bcast, in_=bass.AP(tensor=sscr.tensor, offset=sscr.offset,
                                         ap=[[0, 128], [1, C]]))
            tf_bcast = persist.tile([128, C], F32)
            nc.gpsimd.dma_start(
                out=tf_bcast, in_=bass.AP(tensor=tfscr.tensor, offset=tfscr.offset,
                                          ap=[[0, 128], [1, C]]))

        # ---- Phases D/E/F: QKV, attention, projection (one psum pool) ----
        ev_sb = [persist.tile([128, NQ], mmdt, name=f"evsb{cc}") for cc in range(CT)]
        dinv = persist.tile([128, NQT], F32)
        with (
            tc.tile_pool(name="mmps", bufs=1, space="PSUM") as mmps,
            tc.tile_pool(name="etp", bufs=10) as etp,
            tc.tile_pool(name="drp", bufs=2) as drp,
            tc.tile_pool(name="outp", bufs=2) as outp,
        ):
            # psum tags: st 2x[128,1024] (banks 0-3), ev0/ev1/ds/epi
            # [128,512]-padded (banks 4-7)
            def st_tile():
                return mmps.tile([128, 1024], F32, tag="st", bufs=2, name="st")

            def side_tile(shape, tag, name):
                return mmps.tile(shape, F32, tag=tag, bufs=1, name=name,
                                 padded_shape=[128, 512])

            # QKV chunk emitters: each takes one [128,512] psum slot from a
            # rotating tag and drains on DVE. The stream is interleaved into
            # block 0 of the attention loop so exp starts as soon as stats,
            # Q(0) and K(0) are ready.
            def emit_qm(blk, m, tag):
                slot = side_tile([128, 512], tag, "qp")
                nc.tensor.matmul(
                    slot, w8q[:, :, m * 128:(m + 1) * 128],
                    x8[:, :, blk * 512:(blk + 1) * 512],
                    start=True, stop=True, perf_mode=DR)
                nc.vector.tensor_scalar_add(
                    q8[:, m, blk * 512:(blk + 1) * 512], slot, bqc[m])

            def emit_km(blk, m, tag):
                slot = side_tile([128, 512], tag, "kp")
                nc.tensor.matmul(
                    slot, w8k[:, :, m * 128:(m + 1) * 128],
                    x8[:, :, blk * 512:(blk + 1) * 512],
                    start=True, stop=True, perf_mode=DR)
                nc.vector.tensor_copy(k8[:, m, blk * 512:(blk + 1) * 512], slot)

            def emit_v(p, tag):
                # key pair p: token tiles 2p, 2p+1 -> one [128,512] psum
                # (ktile-major), one drain into v8all
                slot = side_tile([128, 512], tag, "vp")
                for i in range(2):
                    t = 2 * p + i
                    nc.tensor.matmul(
                        slot[:, i * C:(i + 1) * C],
                        x8[:, :, t * 128:(t + 1) * 128], w8v,
                        start=True, stop=True, perf_mode=DR)
                nc.vector.tensor_copy(
                    v8all[:, :, p * C:(p + 1) * C],
                    slot.rearrange("q (i n) -> q i n", i=2))

            # in-loop chunk queue (deadline order: K chunk c by pair 2c-2,
            # V pair p by pair p, Q block b before block b's S matmuls)
            chunkq = []
            for blk in range(1, TOK // 512):
                for m in range(CT):
                    chunkq.append(("k", blk, m))
                chunkq.append(("v", 2 * blk, 0))
                chunkq.append(("v", 2 * blk + 1, 0))
                if blk in (4, 6, 7):
                    qb = {4: 1, 6: 2, 7: 3}[blk]
                    for m in range(CT):
                        chunkq.append(("q", qb, m))
            _rot = [0]
            _dslock = [False]

            def pop_chunks(n):
                for _ in range(n):
                    if not chunkq:
                        return
                    kind, a, b = chunkq.pop(0)
                    tag = "epi" if _dslock[0] else ("epi", "ds")[_rot[0] % 2]
                    _rot[0] += 1
                    if kind == "k":
                        emit_km(a, b, tag)
                    elif kind == "v":
                        emit_v(a, tag)
                    else:
                        emit_qm(a, b, tag)

            # attention + projection per query block
            def emit_qk(nb, pr):
                st = st_tile()
                for sub in range(2):
                    mt = 2 * pr + sub
                    nc.tensor.matmul(
                        st[:, sub * 512:(sub + 1) * 512],
                        k8[:, :, mt * 128:(mt + 1) * 128],
                        q8[:, :, nb * 512:(nb + 1) * 512],
                        start=True, stop=True, perf_mode=DR)
                return st

            def epi_proj(nb):
                for t in range(4 * nb, 4 * nb + 4):
                    yp = side_tile([128, C], "epi", "yp")
                    for kk in range(CT):
                        nc.tensor.matmul(
                            yp, ev_sb[kk][:, t * 128:(t + 1) * 128], wp_t[kk],
                            start=(kk == 0), stop=(kk == CT - 1))
                    yn = outp.tile([128, C], F32, tag="yn")
                    nc.vector.tensor_scalar_mul(yn, yp, dinv[:, t:t + 1])
                    ot = outp.tile([128, C], F32, tag="ot")
                    nc.gpsimd.tensor_add(ot, yn, xkf[t])
                    nc.sync.dma_start(out=out_d[t * 128:(t + 1) * 128, :], in_=ot)

            # prefix: Q(0), K(0), V(0), V(1) on the (free until block 0)
            # ev0/ev1/epi banks, then the S prefill
            emit_qm(0, 0, "ev0")
            emit_qm(0, 1, "ev1")
            emit_km(0, 0, "epi")
            emit_km(0, 1, "ev0")
            emit_v(0, "ev1")
            emit_v(1, "epi")
            sts = [emit_qk(0, 0), emit_qk(0, 1)]
            # residual xn rows (token-major), in place, on Pool (idle here)
            for t in range(NQT):
                nc.gpsimd.tensor_mul(xkf[t], xkf[t], s_bcast)
                nc.gpsimd.tensor_add(xkf[t], xkf[t], tf_bcast)
            pending = None
            for nb in range(NB):
                ev0 = side_tile([128, 512], "ev0", "ev0")
                ev1 = side_tile([128, 512], "ev1", "ev1")
                ds = None
                ets_held = {}
                for pr in range(NPAIR):
                    et = etp.tile([128, 1024], F8, tag="et")
                    nc.scalar.activation(et, sts[pr % 2], AF.Exp, scale=SCALE8,
                                         bias=negc)
                    if nb == 0:
                        pop_chunks(3 if pr < 6 else 2)
                    if pr + 2 < NPAIR:
                        sts[pr % 2] = emit_qk(nb, pr + 2)
                    elif nb + 1 < NB:
                        sts[pr % 2] = emit_qk(nb + 1, pr + 2 - NPAIR)
                    etr = et.rearrange("p (i n) -> p i n", i=2)
                    nc.tensor.matmul(ev0, v8all[:, :, pr * C:pr * C + 128], etr,
                                     start=(pr == 0), stop=(pr == NPAIR - 1),
                                     perf_mode=DR)
                    nc.tensor.matmul(ev1, v8all[:, :, pr * C + 128:(pr + 1) * C],
                                     etr,
                                     start=(pr == 0), stop=(pr == NPAIR - 1),
                                     perf_mode=DR)
                    if nb == 0:
                        # the ds bank hosts QKV chunks for the first half of
                        # block 0; denominator matmuls for pairs 0..7 run
                        # late from held et tiles
                        if pr < 8:
                            ets_held[pr] = etr
                        else:
                            if ds is None:
                                _dslock[0] = True
                                ds = side_tile([32, 512], "ds", "ds")
                            nc.tensor.matmul(ds, ones8, etr,
                                             start=(pr == 8), stop=False,
                                             perf_mode=DR)
                            late = ets_held.pop(pr - 8)
                            nc.tensor.matmul(ds, ones8, late,
                                             start=False, stop=(pr == NPAIR - 1),
                                             perf_mode=DR)
                    else:
                        if ds is None:
                            ds = side_tile([32, 512], "ds", "ds")
                        nc.tensor.matmul(ds, ones8, etr,
                                         start=(pr == 0), stop=(pr == NPAIR - 1),
                                         perf_mode=DR)
                    if pr == 2 and pending is not None:
                        epi_proj(pending)
                        pending = None
                # block end: drain EV (scaled 1/QS) + denominator chain
                nc.vector.tensor_scalar_mul(
                    ev_sb[0][:, nb * 512:(nb + 1) * 512], ev0, 1.0 / QS)
                nc.vector.tensor_scalar_mul(
                    ev_sb[1][:, nb * 512:(nb + 1) * 512], ev1, 1.0 / QS)
                drowt = drp.tile([1, 512], F32, tag="dr")
                nc.vector.tensor_copy(drowt, ds[0:1, :])
                dtp = side_tile([128, 4], "epi", "dtp")
                for j in range(4):
                    nc.tensor.transpose(
                        dtp[:, j:j + 1], drowt[:, j * 128:(j + 1) * 128],
                        ident[0:1, 0:1])
                nc.vector.reciprocal(dinv[:, nb * 4:(nb + 1) * 4], dtp)
                pending = nb
            epi_proj(pending)

    nc.finalize()
    return nc


_NC_CACHE = {}


def _get_nc(use_f32r=True, reps=1):
    key = (use_f32r, reps)
    if key not in _NC_CACHE:
        _NC_CACHE[key] = build_nc(use_f32r, reps)
    return _NC_CACHE[key]


def run(inputs, use_f32r=True, trace=False):
    x = np.ascontiguousarray(np.asarray(inputs["x"], np.float32)).reshape(B, TOK, C)
    common = {
        k: np.ascontiguousarray(np.asarray(inputs[k], np.float32))
        for k in ["Wq", "Wk", "Wv", "Wp", "bq", "bk", "bv", "bp",
                  "gn_gamma", "gn_beta"]
    }
    in_maps = []
    for core in range(N_CORES):
        b, h = core // 2, core % 2
        if h == 0:
            xs = x[b]
        else:
            xs = np.concatenate([x[b][NQ:], x[b][:NQ]], axis=0)
        in_maps.append({"xs": np.ascontiguousarray(xs), **common})

    nc = _get_nc(use_f32r)
    res = run_bass_kernel_spmd(nc, in_maps, list(range(N_CORES)), trace=trace)

    out = np.empty((B, TOK, C), np.float32)
    for core in range(N_CORES):
        b, h = core // 2, core % 2
        out[b, h * NQ:(h + 1) * NQ] = res.results[core]["out"]
    return out.reshape(B, H, W, C), res


def kernel(**inputs):
    out, _ = run(inputs)
    return out
